# revision 1
# baseline (speedup 1.0000x reference)
"""Causal self-attention (B=1, T=2048, C=1024, H=16) on 8 trn2 NeuronCores.

Sharding: tensor-parallel over heads — 2 heads per core. Each core computes
Q/K/V projections for its head pair from the full (replicated) x, interleaved
RoPE, causal softmax attention (the full post-softmax attention matrix is an
output), and its partial contribution to the output projection
(head_slice @ Wp_rows). The host sums the 8 partial y's and assembles att.

Device compute is bf16 matmul inputs / fp32 PSUM accumulation; softmax is
fp32 (exp on ScalarE with fused per-row accumulation).

Layout notes:
 - x is fed transposed (xT, [C, T]) so it serves as the matmul moving operand
   for all three projections.
 - Q/K are produced d-major (QT/KT: [d, T]) with the head-dim columns of
   Wq/Wk permuted to [evens | odds] so interleaved RoPE becomes two
   partition-block copies + mul/add with host-built cos/sin tables (sign of
   sin pre-folded).
 - Attention scores are built twice: q-major (S) for the softmax/att output,
   and k-major (S^T) whose exp (PT, bf16) feeds P@V. Both are bitwise the
   same product, so the two paths are consistent.
 - P@V uses PT as the stationary operand giving Y q-major; an appended
   ones-column of V gives the row-sum r in the same matmuls. Y is normalized
   per-partition by 1/r, PE-transposed back to d-major, and the output
   projection runs as a single K=128 matmul per row block.
"""

import math

import numpy as np

B, T, C, H = 1, 2048, 1024, 16
D = C // H  # 64
NCORES = 8
HPC = H // NCORES  # heads per core = 2
NB = T // 128  # 16 row blocks
SCALE = 1.0 / math.sqrt(D)
MASK_NEG = -30000.0

_CACHE = {}


def _build_nc():
    import concourse.mybir as mybir
    import concourse.tile as tile
    from concourse import bacc

    f32 = mybir.dt.float32
    bf16 = mybir.dt.bfloat16
    EXP = mybir.ActivationFunctionType.Exp

    nc = bacc.Bacc("TRN2", target_bir_lowering=False, debug=False, num_devices=NCORES)

    xT = nc.dram_tensor("xT", [C, T], bf16, kind="ExternalInput").ap()
    wq = nc.dram_tensor("wq", [C, 128], bf16, kind="ExternalInput").ap()
    wk = nc.dram_tensor("wk", [C, 128], bf16, kind="ExternalInput").ap()
    wv = nc.dram_tensor("wv", [C, 128], bf16, kind="ExternalInput").ap()
    wp = nc.dram_tensor("wp", [128, C], bf16, kind="ExternalInput").ap()
    bq = nc.dram_tensor("bq", [128, 1], f32, kind="ExternalInput").ap()
    bk = nc.dram_tensor("bk", [128, 1], f32, kind="ExternalInput").ap()
    cosT = nc.dram_tensor("cosT", [128, T], bf16, kind="ExternalInput").ap()
    sinT = nc.dram_tensor("sinT", [128, T], bf16, kind="ExternalInput").ap()
    maskA = nc.dram_tensor("maskA", [128, 128], bf16, kind="ExternalInput").ap()
    maskAT = nc.dram_tensor("maskAT", [128, 128], bf16, kind="ExternalInput").ap()
    ident = nc.dram_tensor("ident", [128, 128], bf16, kind="ExternalInput").ap()

    att = nc.dram_tensor("att", [HPC, T, T], f32, kind="ExternalOutput").ap()
    y = nc.dram_tensor("y", [T, C], f32, kind="ExternalOutput").ap()

    with tile.TileContext(nc) as tc:
        with tc.tile_pool(name="persist", bufs=1) as persist:
            # ---- resident SBUF tensors ----
            xT_sb = persist.tile([128, 8, T], bf16, tag="xT")
            wq_sb = persist.tile([128, 8, 128], bf16, tag="wq")
            wk_sb = persist.tile([128, 8, 128], bf16, tag="wk")
            wv_sb = persist.tile([128, 8, 128], bf16, tag="wv")
            wp_sb = persist.tile([128, C], bf16, tag="wp")
            bq_sb = persist.tile([128, 1], f32, tag="bq")
            bk_sb = persist.tile([128, 1], f32, tag="bk")
            cos_sb = persist.tile([128, T], bf16, tag="cos")
            sin_sb = persist.tile([128, T], bf16, tag="sin")
            mA_sb = persist.tile([128, 128], bf16, tag="mA")
            mAT_sb = persist.tile([128, 128], bf16, tag="mAT")
            id_sb = persist.tile([128, 128], bf16, tag="id")
            qt_sb = persist.tile([128, T], bf16, tag="qt")
            kt_sb = persist.tile([128, T], bf16, tag="kt")
            v_sb = persist.tile([128, NB, 2 * (D + 1)], bf16, tag="v")
            yt_sb = persist.tile([128, T], bf16, tag="yt")

            nc.sync.dma_start(out=xT_sb, in_=xT.rearrange("(c p) t -> p c t", p=128))
            nc.sync.dma_start(out=wq_sb, in_=wq.rearrange("(c p) d -> p c d", p=128))
            nc.sync.dma_start(out=wk_sb, in_=wk.rearrange("(c p) d -> p c d", p=128))
            nc.sync.dma_start(out=wv_sb, in_=wv.rearrange("(c p) d -> p c d", p=128))
            nc.sync.dma_start(out=wp_sb, in_=wp)
            nc.sync.dma_start(out=bq_sb, in_=bq)
            nc.sync.dma_start(out=bk_sb, in_=bk)
            nc.sync.dma_start(out=cos_sb, in_=cosT)
            nc.sync.dma_start(out=sin_sb, in_=sinT)
            nc.sync.dma_start(out=mA_sb, in_=maskA)
            nc.sync.dma_start(out=mAT_sb, in_=maskAT)
            nc.sync.dma_start(out=id_sb, in_=ident)

            # ones columns of the augmented V (for the P@V row-sum trick)
            nc.vector.memset(v_sb[:, :, D], 1.0)
            nc.vector.memset(v_sb[:, :, 2 * D + 1], 1.0)

            # ================= QKV projections =================
            with (
                tc.tile_pool(name="psqk", bufs=2, space="PSUM") as psqk,
                tc.tile_pool(name="psv", bufs=2, space="PSUM") as psv,
                tc.tile_pool(name="rope_tmp", bufs=1) as ropep,
            ):
                for dst, w_sb, b_sb in ((qt_sb, wq_sb, bq_sb), (kt_sb, wk_sb, bk_sb)):
                    for half in range(2):
                        ps = psqk.tile([128, 1024], f32, tag="psqk")
                        for nch in range(2):
                            sl = slice(nch * 512, (nch + 1) * 512)
                            for cc in range(8):
                                nc.tensor.matmul(
                                    ps[:, sl],
                                    lhsT=w_sb[:, cc, :],
                                    rhs=xT_sb[:, cc, half * 1024 + nch * 512:half * 1024 + (nch + 1) * 512],
                                    start=(cc == 0),
                                    stop=(cc == 7),
                                )
                        nc.vector.tensor_scalar_add(
                            dst[:, half * 1024:(half + 1) * 1024], ps, b_sb
                        )

                for kb in range(NB):
                    pv = psv.tile([128, 128], f32, tag="psv")
                    for cc in range(8):
                        nc.tensor.matmul(
                            pv,
                            lhsT=xT_sb[:, cc, kb * 128:(kb + 1) * 128],
                            rhs=wv_sb[:, cc, :],
                            start=(cc == 0),
                            stop=(cc == 7),
                        )
                    nc.vector.tensor_copy(v_sb[:, kb, 0:D], pv[:, 0:D])
                    nc.vector.tensor_copy(v_sb[:, kb, D + 1:2 * D + 1], pv[:, D:2 * D])

                # ---- RoPE on QT/KT (in permuted-even/odd layout) ----
                for tgt in (qt_sb, kt_sb):
                    tmp = ropep.tile([128, T], bf16, tag="rope")
                    for h in range(2):
                        o = h * 64
                        nc.vector.tensor_copy(tmp[o:o + 32, :], tgt[o + 32:o + 64, :])
                        nc.vector.tensor_copy(tmp[o + 32:o + 64, :], tgt[o:o + 32, :])
                    nc.vector.tensor_mul(tgt, tgt, cos_sb)
                    nc.vector.tensor_mul(tmp, tmp, sin_sb)
                    nc.vector.tensor_add(tgt, tgt, tmp)

            # ================= attention per head =================
            for h in range(HPC):
                hs = slice(h * 64, (h + 1) * 64)

                # ---- phase A: q-major scores -> softmax -> att out ----
                with (
                    tc.tile_pool(name=f"psA{h}", bufs=2, space="PSUM") as psA,
                    tc.tile_pool(name=f"pA{h}", bufs=2) as pA,
                    tc.tile_pool(name=f"attA{h}", bufs=3) as attA,
                    tc.tile_pool(name=f"rA{h}", bufs=6) as rA,
                ):
                    for qb in range(NB):
                        klen = (qb + 1) * 128
                        ps = psA.tile([128, T], f32, tag="sA")
                        for nch in range((klen + 511) // 512):
                            c0, c1 = nch * 512, min((nch + 1) * 512, klen)
                            has_diag = c1 == klen
                            nc.tensor.matmul(
                                ps[:, c0:c1],
                                lhsT=qt_sb[hs, qb * 128:(qb + 1) * 128],
                                rhs=kt_sb[hs, c0:c1],
                                start=True,
                                stop=not has_diag,
                            )
                            if has_diag:
                                nc.tensor.matmul(
                                    ps[:, klen - 128:klen],
                                    lhsT=id_sb,
                                    rhs=mA_sb,
                                    start=False,
                                    stop=True,
                                )
                        p_t = pA.tile([128, T], f32, tag="p")
                        r_t = rA.tile([128, 1], f32, tag="r")
                        nc.scalar.activation(
                            p_t[:, 0:klen], ps[:, 0:klen], EXP, scale=SCALE,
                            accum_out=r_t,
                        )
                        inv_t = rA.tile([128, 1], f32, tag="inv")
                        nc.vector.reciprocal(inv_t, r_t)
                        a_t = attA.tile([128, T], f32, tag="att")
                        nc.vector.tensor_scalar_mul(a_t[:, 0:klen], p_t[:, 0:klen], inv_t)
                        nc.sync.dma_start(
                            out=att[h, qb * 128:(qb + 1) * 128, 0:klen],
                            in_=a_t[:, 0:klen],
                        )

                # ---- phase B: k-major scores -> PT -> Y = P@V -> YT ----
                with (
                    tc.tile_pool(name=f"psB{h}", bufs=2, space="PSUM") as psB,
                    tc.tile_pool(name=f"psY{h}", bufs=2, space="PSUM") as psY,
                    tc.tile_pool(name=f"psT{h}", bufs=2, space="PSUM") as psT,
                    tc.tile_pool(name=f"ptp{h}", bufs=1) as ptp,
                    tc.tile_pool(name=f"ysc{h}", bufs=2) as ysc,
                    tc.tile_pool(name=f"rB{h}", bufs=4) as rB,
                ):
                    pt_strips = []
                    for qb in range(NB):
                        # strip kb == qb: S^T over q in [qb*128, T)
                        qlen = T - qb * 128
                        pt_t = ptp.tile([128, qlen], bf16, tag=f"pt{qb}")
                        for sch in range((qlen + 1023) // 1024):
                            s0 = sch * 1024
                            s1 = min(s0 + 1024, qlen)
                            pb = psB.tile([128, 1024], f32, tag="sB")
                            for nch in range((s1 - s0 + 511) // 512):
                                c0 = s0 + nch * 512
                                c1 = min(c0 + 512, s1)
                                has_diag = c0 == 0
                                nc.tensor.matmul(
                                    pb[:, c0 - s0:c1 - s0],
                                    lhsT=kt_sb[hs, qb * 128:(qb + 1) * 128],
                                    rhs=qt_sb[hs, qb * 128 + c0:qb * 128 + c1],
                                    start=True,
                                    stop=not has_diag,
                                )
                                if has_diag:
                                    nc.tensor.matmul(
                                        pb[:, 0:128],
                                        lhsT=id_sb,
                                        rhs=mAT_sb,
                                        start=False,
                                        stop=True,
                                    )
                            nc.scalar.activation(
                                pt_t[:, s0:s1], pb[:, 0:s1 - s0], EXP, scale=SCALE
                            )
                        pt_strips.append(pt_t)

                        # Y[qb] = sum_kb PT[kb][:, qb].T @ [V|1]
                        py = psY.tile([128, D + 1], f32, tag="y")
                        for kb in range(qb + 1):
                            nc.tensor.matmul(
                                py,
                                lhsT=pt_strips[kb][:, (qb - kb) * 128:(qb - kb + 1) * 128],
                                rhs=v_sb[:, kb, h * (D + 1):(h + 1) * (D + 1)],
                                start=(kb == 0),
                                stop=(kb == qb),
                            )
                        rinv = rB.tile([128, 1], f32, tag="rinv")
                        nc.vector.reciprocal(rinv, py[:, D:D + 1])
                        ysb = ysc.tile([128, D], bf16, tag="ysb")
                        nc.vector.tensor_scalar_mul(ysb, py[:, 0:D], rinv)
                        pt2 = psT.tile([64, 128], bf16, tag="tp")
                        nc.tensor.transpose(pt2, ysb, id_sb)
                        nc.vector.tensor_copy(
                            yt_sb[hs, qb * 128:(qb + 1) * 128], pt2
                        )

            # ================= output projection =================
            with (
                tc.tile_pool(name="psP", bufs=4, space="PSUM") as psP,
                tc.tile_pool(name="yo", bufs=4) as yop,
            ):
                for tb in range(NB):
                    for nch in range(2):
                        pp = psP.tile([128, 512], f32, tag="pp")
                        nc.tensor.matmul(
                            pp,
                            lhsT=yt_sb[:, tb * 128:(tb + 1) * 128],
                            rhs=wp_sb[:, nch * 512:(nch + 1) * 512],
                            start=True,
                            stop=True,
                        )
                        yo = yop.tile([128, 512], f32, tag="yo")
                        nc.vector.tensor_copy(yo, pp)
                        nc.sync.dma_start(
                            out=y[tb * 128:(tb + 1) * 128, nch * 512:(nch + 1) * 512],
                            in_=yo,
                        )

    nc.compile()
    return nc


def _perm_cols(h):
    base = h * D
    return np.concatenate([base + np.arange(0, D, 2), base + np.arange(1, D, 2)])


def _rope_tables():
    # permuted layout: per head, partitions [0:32) = even dims, [32:64) = odd
    div = np.exp(np.arange(0, D, 2, dtype=np.float64) * (-math.log(10000.0) / D))
    ang = np.arange(T, dtype=np.float64)[None, :] * div[:, None]  # [32, T]
    cos32 = np.cos(ang)
    sin32 = np.sin(ang)
    cos64 = np.concatenate([cos32, cos32], axis=0)  # [64, T]
    sin64 = np.concatenate([-sin32, sin32], axis=0)  # evens get -sin
    cosP = np.concatenate([cos64, cos64], axis=0).astype(np.float32)  # [128, T]
    sinP = np.concatenate([sin64, sin64], axis=0).astype(np.float32)
    return cosP, sinP


def kernel(x, Wq, bq, Wk, bk, Wv, bv, Wp, bp):
    import ml_dtypes
    from concourse.bass_utils import run_bass_kernel_spmd

    bf16 = ml_dtypes.bfloat16

    if "nc" not in _CACHE:
        _CACHE["nc"] = _build_nc()
    nc = _CACHE["nc"]

    x = np.asarray(x, np.float32)
    Wq, Wk, Wv, Wp = (np.asarray(w, np.float32) for w in (Wq, Wk, Wv, Wp))
    bq, bk, bv, bp = (np.asarray(b, np.float32) for b in (bq, bk, bv, bp))

    xT = np.ascontiguousarray(x[0].T).astype(bf16)
    cosP, sinP = _rope_tables()
    qk = np.arange(128)
    maskA = np.where(qk[None, :] <= qk[:, None], 0.0, MASK_NEG).astype(np.float32)

    common = {
        "xT": xT,
        "cosT": cosP.astype(bf16),
        "sinT": sinP.astype(bf16),
        "maskA": maskA.astype(bf16),
        "maskAT": np.ascontiguousarray(maskA.T).astype(bf16),
        "ident": np.eye(128, dtype=np.float32).astype(bf16),
    }
    in_maps = []
    for c in range(NCORES):
        cols = np.concatenate([_perm_cols(2 * c), _perm_cols(2 * c + 1)])
        rows = slice(2 * c * D, (2 * c + 2) * D)
        in_maps.append(
            dict(
                common,
                wq=np.ascontiguousarray(Wq[:, cols]).astype(bf16),
                wk=np.ascontiguousarray(Wk[:, cols]).astype(bf16),
                wv=np.ascontiguousarray(Wv[:, rows]).astype(bf16),
                wp=np.ascontiguousarray(Wp[rows, :]).astype(bf16),
                bq=np.ascontiguousarray(bq[cols]).reshape(128, 1),
                bk=np.ascontiguousarray(bk[cols]).reshape(128, 1),
            )
        )

    res = run_bass_kernel_spmd(nc, in_maps, core_ids=list(range(NCORES)))

    att_full = np.zeros((B, H, T, T), np.float32)
    y_full = np.zeros((T, C), np.float32)
    for c in range(NCORES):
        out = res.results[c]
        y_full += out["y"]
        for hl in range(HPC):
            a = out["att"][hl]
            dst = att_full[0, 2 * c + hl]
            for qb in range(NB):
                klen = (qb + 1) * 128
                dst[qb * 128:(qb + 1) * 128, 0:klen] = a[qb * 128:(qb + 1) * 128, 0:klen]

    # biases folded on host: V bias contributes (sum_k att)=1 times bv per row
    y_full += bv @ Wp + bp
    return y_full.reshape(B, T, C), att_full


if __name__ == "__main__":
    rng = np.random.default_rng(0)
    s = 1.0 / math.sqrt(C)
    ins = {
        "x": rng.standard_normal((B, T, C), np.float32),
        "Wq": rng.standard_normal((C, C), np.float32) * s,
        "bq": np.zeros(C, np.float32),
        "Wk": rng.standard_normal((C, C), np.float32) * s,
        "bk": np.zeros(C, np.float32),
        "Wv": rng.standard_normal((C, C), np.float32) * s,
        "bv": np.zeros(C, np.float32),
        "Wp": rng.standard_normal((C, C), np.float32) * s,
        "bp": np.zeros(C, np.float32),
    }
    out_y, out_att = kernel(**ins)
    print("y", out_y.shape, out_y.dtype, "att", out_att.shape, out_att.dtype)


# revision 24
# speedup vs baseline: 1.4272x; 1.4272x over previous
"""Causal self-attention (B=1, T=2048, C=1024, H=16) on 8 trn2 NeuronCores.

Sharding: tensor-parallel over heads — 2 heads per core. Each core computes
Q/K/V projections for its head pair from the full (replicated) x, interleaved
RoPE, causal softmax attention (the full post-softmax attention matrix is an
output), and its partial contribution to the output projection
(head_slice @ Wp_rows). The host sums the 8 partial y's and assembles att.

Device compute is bf16 matmul inputs / fp32 PSUM accumulation; softmax is
fp32 (exp on ScalarE with fused per-row accumulation).

Layout notes:
 - x is fed transposed (xT, [C, T]) so it serves as the matmul moving operand
   for all three projections.
 - Q/K are produced d-major (QT/KT: [d, T]) with the head-dim columns of
   Wq/Wk permuted to [evens | odds] so interleaved RoPE becomes two
   partition-block copies + mul/add with host-built cos/sin tables (sign of
   sin pre-folded).
 - Attention scores are built twice: q-major (S) for the softmax/att output,
   and k-major (S^T) whose exp (PT, bf16) feeds P@V. Both are bitwise the
   same product, so the two paths are consistent.
 - P@V uses PT as the stationary operand giving Y q-major; an appended
   ones-column of V gives the row-sum r in the same matmuls. Y is normalized
   per-partition by 1/r, PE-transposed back to d-major, and the output
   projection runs as a single K=128 matmul per row block.
"""

import math

import numpy as np

B, T, C, H = 1, 2048, 1024, 16
D = C // H  # 64
NCORES = 8
HPC = H // NCORES  # heads per core = 2
NB = T // 128  # 16 row blocks
SCALE = 1.0 / math.sqrt(D)
MASK_NEG = -30000.0

_CACHE = {}


def _build_nc(ablate=()):
    import concourse.mybir as mybir
    import concourse.tile as tile
    from concourse import bacc

    f32 = mybir.dt.float32
    bf16 = mybir.dt.bfloat16
    EXP = mybir.ActivationFunctionType.Exp

    nc = bacc.Bacc("TRN2", target_bir_lowering=False, debug=False, num_devices=NCORES)

    xT = nc.dram_tensor("xT", [C, T], bf16, kind="ExternalInput").ap()
    wq = nc.dram_tensor("wq", [C, 128], bf16, kind="ExternalInput").ap()
    wk = nc.dram_tensor("wk", [C, 128], bf16, kind="ExternalInput").ap()
    wv = nc.dram_tensor("wv", [C, 128], bf16, kind="ExternalInput").ap()
    wp = nc.dram_tensor("wp", [128, C], bf16, kind="ExternalInput").ap()
    bq = nc.dram_tensor("bq", [128, 1], f32, kind="ExternalInput").ap()
    bk = nc.dram_tensor("bk", [128, 1], f32, kind="ExternalInput").ap()
    cosT = nc.dram_tensor("cosT", [128, T], bf16, kind="ExternalInput").ap()
    sinT = nc.dram_tensor("sinT", [128, T], bf16, kind="ExternalInput").ap()
    maskA = nc.dram_tensor("maskA", [128, 128], bf16, kind="ExternalInput").ap()
    maskAT = nc.dram_tensor("maskAT", [128, 128], bf16, kind="ExternalInput").ap()
    ident = nc.dram_tensor("ident", [128, 128], bf16, kind="ExternalInput").ap()
    identf = nc.dram_tensor("identf", [128, 128], f32, kind="ExternalInput").ap()

    att = nc.dram_tensor("att", [HPC, T, T], f32, kind="ExternalOutput").ap()
    y = nc.dram_tensor("y", [T, C], f32, kind="ExternalOutput").ap()

    with tile.TileContext(nc) as tc:
        with (
            tc.tile_pool(name="persist", bufs=1) as persist,
            tc.tile_pool(name="sp", bufs=3, space="PSUM") as sp,
            tc.tile_pool(name="psY", bufs=2, space="PSUM") as psY,
            tc.tile_pool(name="attA", bufs=4) as attA,
            tc.tile_pool(name="ysc", bufs=2) as ysc,
            tc.tile_pool(name="small", bufs=8) as small,
            tc.tile_pool(name="yo", bufs=2) as yop,
            tc.tile_pool(name="rope_tmp", bufs=2) as ropep,
        ):
            # ---- resident SBUF tensors ----
            xT_sb = persist.tile([128, 8, T], bf16, tag="xT")
            wq_sb = persist.tile([128, 8, 128], bf16, tag="wq")
            wk_sb = persist.tile([128, 8, 128], bf16, tag="wk")
            wv_sb = persist.tile([128, 8, 128], bf16, tag="wv")
            wp_sb = persist.tile([128, C], bf16, tag="wp")
            bq_sb = persist.tile([128, 1], f32, tag="bq")
            bk_sb = persist.tile([128, 1], f32, tag="bk")
            cos_sb = persist.tile([128, T], bf16, tag="cos")
            sin_sb = persist.tile([128, T], bf16, tag="sin")
            mA_sb = persist.tile([128, 128], bf16, tag="mA")
            mAT_sb = persist.tile([128, 128], bf16, tag="mAT")
            id_sb = persist.tile([128, 128], bf16, tag="id")
            idf_sb = persist.tile([128, 128], f32, tag="idf")
            qt_sb = persist.tile([128, T], bf16, tag="qt")
            kt_sb = persist.tile([128, T], bf16, tag="kt")
            v_sb = persist.tile([128, NB, 2 * (D + 1)], bf16, tag="v")
            yt_sb = persist.tile([128, T], bf16, tag="yt")

            # warm up the ACT exp table while input DMAs run
            wu = small.tile([1, 8], f32, tag="wu")
            wu2 = small.tile([1, 8], f32, tag="wu2")
            nc.vector.memset(wu, 0.0)
            nc.scalar.activation(wu2, wu, EXP)

            # weights for Q/K first — they gate the first matmuls
            nc.sync.dma_start(out=wq_sb, in_=wq.rearrange("(c p) d -> p c d", p=128))
            nc.sync.dma_start(out=wk_sb, in_=wk.rearrange("(c p) d -> p c d", p=128))
            nc.sync.dma_start(out=bq_sb, in_=bq)
            nc.sync.dma_start(out=bk_sb, in_=bk)
            nc.sync.dma_start(out=cos_sb, in_=cosT)
            nc.sync.dma_start(out=sin_sb, in_=sinT)
            xTr = xT.rearrange("(c p) t -> p c t", p=128)
            for cc in range(8):
                nc.sync.dma_start(out=xT_sb[:, cc, :], in_=xTr[:, cc, :])
            nc.sync.dma_start(out=mA_sb, in_=maskA)
            nc.sync.dma_start(out=mAT_sb, in_=maskAT)
            nc.sync.dma_start(out=id_sb, in_=ident)
            nc.sync.dma_start(out=idf_sb, in_=identf)
            nc.sync.dma_start(out=wv_sb, in_=wv.rearrange("(c p) d -> p c d", p=128))
            nc.sync.dma_start(out=wp_sb, in_=wp)

            # ones columns of the augmented V (for the P@V row-sum trick)
            nc.vector.memset(v_sb[:, :, D], 1.0)
            nc.vector.memset(v_sb[:, :, 2 * D + 1], 1.0)

            # ========= Q/K projections + RoPE, in 512-col t-chunks so
            # ========= attention on early row-blocks starts ASAP
            for tch in range(4):
                t0 = tch * 512
                ts = slice(t0, t0 + 512)
                for dst, w_sb, b_sb in ((qt_sb, wq_sb, bq_sb), (kt_sb, wk_sb, bk_sb)):
                    ps = sp.tile([128, 1024], f32, tag="s")
                    for cc in range(8):
                        nc.tensor.matmul(
                            ps[:, 0:512],
                            lhsT=w_sb[:, cc, :],
                            rhs=xT_sb[:, cc, ts],
                            start=(cc == 0),
                            stop=(cc == 7),
                        )
                    # bias-add + bf16 cast on ScalarE — ACT is idle during
                    # startup and this keeps the PSUM slot rotation off DVE
                    nc.scalar.add(dst[:, ts], ps[:, 0:512], b_sb)

                for tgt in (qt_sb, kt_sb):
                    tmp = ropep.tile([128, 512], bf16, tag="rope")
                    for h in range(2):
                        o = h * 64
                        nc.vector.tensor_copy(tmp[o:o + 32, :], tgt[o + 32:o + 64, ts])
                        nc.vector.tensor_copy(tmp[o + 32:o + 64, :], tgt[o:o + 32, ts])
                    nc.vector.tensor_mul(tgt[:, ts], tgt[:, ts], cos_sb[:, ts])
                    nc.vector.tensor_mul(tmp, tmp, sin_sb[:, ts])
                    nc.vector.tensor_add(tgt[:, ts], tgt[:, ts], tmp)

            def emit_V(kb):
                if 'V' in ablate:
                    return
                pv = sp.tile([128, 1024], f32, tag="s")
                for cc in range(8):
                    nc.tensor.matmul(
                        pv[:, 0:128],
                        lhsT=xT_sb[:, cc, kb * 128:(kb + 1) * 128],
                        rhs=wv_sb[:, cc, :],
                        start=(cc == 0),
                        stop=(cc == 7),
                    )
                nc.vector.tensor_copy(
                    v_sb[:, kb, :].rearrange("p (g x) -> p g x", g=2)[:, :, 0:D],
                    pv[:, 0:128].rearrange("p (g x) -> p g x", g=2),
                )

            def emit_A(h, qb):
                if 'A' in ablate:
                    return
                hs = slice(h * 64, (h + 1) * 64)
                klen = (qb + 1) * 128
                a_t = attA.tile([128, T], f32, tag="att")
                rparts = []
                for ch in range((klen + 1023) // 1024):
                    c0, c1 = ch * 1024, min(ch * 1024 + 1024, klen)
                    ps = sp.tile([128, 1024], f32, tag="s")
                    for m0 in range(c0, c1, 512):
                        m1 = min(m0 + 512, c1)
                        last = m1 == klen
                        nc.tensor.matmul(
                            ps[:, m0 - c0:m1 - c0],
                            lhsT=qt_sb[hs, qb * 128:(qb + 1) * 128],
                            rhs=kt_sb[hs, m0:m1],
                            start=True,
                            stop=not last,
                        )
                        if last:
                            nc.tensor.matmul(
                                ps[:, klen - 128 - c0:klen - c0],
                                lhsT=id_sb,
                                rhs=mA_sb,
                                start=False,
                                stop=True,
                            )
                    r_c = small.tile([128, 1], f32, tag="r")
                    nc.scalar.activation(
                        a_t[:, c0:c1], ps[:, 0:c1 - c0], EXP, scale=SCALE,
                        accum_out=r_c,
                    )
                    rparts.append(r_c)
                if len(rparts) == 2:
                    rsum = small.tile([128, 1], f32, tag="r")
                    nc.vector.tensor_add(rsum, rparts[0], rparts[1])
                else:
                    rsum = rparts[0]
                inv_t = small.tile([128, 1], f32, tag="inv")
                nc.vector.reciprocal(inv_t, rsum)
                # normalization split across GpSimd (idle) and DVE so the
                # halves run in parallel and the latency on the critical
                # path is halved
                if klen > 512:
                    half = (klen // 2 + 127) & ~127
                    nc.gpsimd.tensor_scalar_mul(
                        a_t[:, 0:half], a_t[:, 0:half], inv_t
                    )
                    nc.vector.tensor_scalar_mul(
                        a_t[:, half:klen], a_t[:, half:klen], inv_t
                    )
                else:
                    nc.vector.tensor_scalar_mul(a_t[:, 0:klen], a_t[:, 0:klen], inv_t)
                if 'adma' not in ablate:
                    nc.sync.dma_start(
                        out=att[h, qb * 128:(qb + 1) * 128, 0:klen],
                        in_=a_t[:, 0:klen],
                    )

            def emit_B(h, qb, pt_strips, ptp):
                if 'B' in ablate:
                    return
                hs = slice(h * 64, (h + 1) * 64)
                qlen = T - qb * 128
                pt_t = ptp.tile([128, qlen], bf16, tag=f"pt{qb}")
                for sch in range((qlen + 1023) // 1024):
                    s0, s1 = sch * 1024, min(sch * 1024 + 1024, qlen)
                    pb = sp.tile([128, 1024], f32, tag="s")
                    for m0 in range(s0, s1, 512):
                        m1 = min(m0 + 512, s1)
                        first = m0 == 0
                        nc.tensor.matmul(
                            pb[:, m0 - s0:m1 - s0],
                            lhsT=kt_sb[hs, qb * 128:(qb + 1) * 128],
                            rhs=qt_sb[hs, qb * 128 + m0:qb * 128 + m1],
                            start=True,
                            stop=not first,
                        )
                        if first:
                            nc.tensor.matmul(
                                pb[:, 0:128],
                                lhsT=id_sb,
                                rhs=mAT_sb,
                                start=False,
                                stop=True,
                            )
                    nc.scalar.activation(
                        pt_t[:, s0:s1], pb[:, 0:s1 - s0], EXP, scale=SCALE
                    )
                pt_strips.append(pt_t)

                # Y[qb] = sum_kb PT[kb][:, qb].T @ [V|1]; kb descending so the
                # freshest strip (kb==qb, still in flight on ACT) comes first
                # and the PSUM accumulator isn't held across its exp latency.
                # The tile has spare columns used as the transpose target —
                # the two accumulation groups are sequential in the same bank.
                py = psY.tile([128, D + 1 + 128], f32, tag="y")
                for i, kb in enumerate(range(qb, -1, -1)):
                    nc.tensor.matmul(
                        py[:, 0:D + 1],
                        lhsT=pt_strips[kb][:, (qb - kb) * 128:(qb - kb + 1) * 128],
                        rhs=v_sb[:, kb, h * (D + 1):(h + 1) * (D + 1)],
                        start=(i == 0),
                        stop=(i == qb),
                    )
                rinv = small.tile([128, 1], f32, tag="rinv")
                nc.vector.reciprocal(rinv, py[:, D:D + 1])
                ysb = ysc.tile([128, D], f32, tag="ysb")
                nc.vector.tensor_scalar_mul(ysb, py[:, 0:D], rinv)
                nc.tensor.transpose(py[0:D, D + 1:D + 1 + 128], ysb, idf_sb)
                nc.vector.tensor_copy(
                    yt_sb[hs, qb * 128:(qb + 1) * 128], py[0:D, D + 1:D + 1 + 128]
                )

            def emit_proj(tb):
                if 'P' in ablate:
                    return
                yo = yop.tile([128, 1024], f32, tag="yo")
                for nch in range(2):
                    pp = sp.tile([128, 1024], f32, tag="s")
                    nc.tensor.matmul(
                        pp[:, 0:512],
                        lhsT=yt_sb[:, tb * 128:(tb + 1) * 128],
                        rhs=wp_sb[:, nch * 512:(nch + 1) * 512],
                        start=True,
                        stop=True,
                    )
                    nc.vector.tensor_copy(yo[:, nch * 512:(nch + 1) * 512], pp[:, 0:512])
                nc.sync.dma_start(out=y[tb * 128:(tb + 1) * 128, :], in_=yo)

            # ==== interleaved attention (both heads) / V / projection ====
            with (
                tc.tile_pool(name="ptp0", bufs=1) as ptp0,
                tc.tile_pool(name="ptp1", bufs=1) as ptp1,
            ):
                strips = {0: [], 1: []}
                ptps = {0: ptp0, 1: ptp1}
                # A-order: small blocks first (only need the first RoPE'd
                # t-half), then largest-first to front-load the att DMA
                a_order = list(range(8)) + list(range(NB - 1, 7, -1))
                # proj lags 4 iterations so its PSUM-slot reuse never couples
                # the next iteration's work to this iteration's tail
                PROJ_LAG = 2
                emit_V(0)
                for i in range(NB):
                    emit_A(0, a_order[i])
                    emit_A(1, a_order[i])
                    if i + 1 < NB:
                        emit_V(i + 1)
                    emit_B(0, i, strips[0], ptps[0])
                    emit_B(1, i, strips[1], ptps[1])
                    if i >= PROJ_LAG:
                        emit_proj(i - PROJ_LAG)
                for tb in range(NB - PROJ_LAG, NB):
                    emit_proj(tb)

    nc.compile()
    return nc


def _perm_cols(h):
    base = h * D
    return np.concatenate([base + np.arange(0, D, 2), base + np.arange(1, D, 2)])


def _rope_tables():
    # permuted layout: per head, partitions [0:32) = even dims, [32:64) = odd
    div = np.exp(np.arange(0, D, 2, dtype=np.float64) * (-math.log(10000.0) / D))
    ang = np.arange(T, dtype=np.float64)[None, :] * div[:, None]  # [32, T]
    cos32 = np.cos(ang)
    sin32 = np.sin(ang)
    cos64 = np.concatenate([cos32, cos32], axis=0)  # [64, T]
    sin64 = np.concatenate([-sin32, sin32], axis=0)  # evens get -sin
    cosP = np.concatenate([cos64, cos64], axis=0).astype(np.float32)  # [128, T]
    sinP = np.concatenate([sin64, sin64], axis=0).astype(np.float32)
    return cosP, sinP


def kernel(x, Wq, bq, Wk, bk, Wv, bv, Wp, bp):
    import ml_dtypes
    from concourse.bass_utils import run_bass_kernel_spmd

    bf16 = ml_dtypes.bfloat16

    if "nc" not in _CACHE:
        _CACHE["nc"] = _build_nc()
    nc = _CACHE["nc"]

    x = np.asarray(x, np.float32)
    Wq, Wk, Wv, Wp = (np.asarray(w, np.float32) for w in (Wq, Wk, Wv, Wp))
    bq, bk, bv, bp = (np.asarray(b, np.float32) for b in (bq, bk, bv, bp))

    xT = np.ascontiguousarray(x[0].T).astype(bf16)
    cosP, sinP = _rope_tables()
    qk = np.arange(128)
    maskA = np.where(qk[None, :] <= qk[:, None], 0.0, MASK_NEG).astype(np.float32)

    common = {
        "xT": xT,
        "cosT": cosP.astype(bf16),
        "sinT": sinP.astype(bf16),
        "maskA": maskA.astype(bf16),
        "maskAT": np.ascontiguousarray(maskA.T).astype(bf16),
        "ident": np.eye(128, dtype=np.float32).astype(bf16),
        "identf": np.eye(128, dtype=np.float32),
    }
    in_maps = []
    for c in range(NCORES):
        cols = np.concatenate([_perm_cols(2 * c), _perm_cols(2 * c + 1)])
        rows = slice(2 * c * D, (2 * c + 2) * D)
        in_maps.append(
            dict(
                common,
                wq=np.ascontiguousarray(Wq[:, cols]).astype(bf16),
                wk=np.ascontiguousarray(Wk[:, cols]).astype(bf16),
                wv=np.ascontiguousarray(Wv[:, rows]).astype(bf16),
                wp=np.ascontiguousarray(Wp[rows, :]).astype(bf16),
                bq=np.ascontiguousarray(bq[cols]).reshape(128, 1),
                bk=np.ascontiguousarray(bk[cols]).reshape(128, 1),
            )
        )

    res = run_bass_kernel_spmd(nc, in_maps, core_ids=list(range(NCORES)))

    att_full = np.zeros((B, H, T, T), np.float32)
    y_full = np.zeros((T, C), np.float32)
    for c in range(NCORES):
        out = res.results[c]
        y_full += out["y"]
        for hl in range(HPC):
            a = out["att"][hl]
            dst = att_full[0, 2 * c + hl]
            for qb in range(NB):
                klen = (qb + 1) * 128
                dst[qb * 128:(qb + 1) * 128, 0:klen] = a[qb * 128:(qb + 1) * 128, 0:klen]

    # biases folded on host: V bias contributes (sum_k att)=1 times bv per row
    y_full += bv @ Wp + bp
    return y_full.reshape(B, T, C), att_full


if __name__ == "__main__":
    rng = np.random.default_rng(0)
    s = 1.0 / math.sqrt(C)
    ins = {
        "x": rng.standard_normal((B, T, C), np.float32),
        "Wq": rng.standard_normal((C, C), np.float32) * s,
        "bq": np.zeros(C, np.float32),
        "Wk": rng.standard_normal((C, C), np.float32) * s,
        "bk": np.zeros(C, np.float32),
        "Wv": rng.standard_normal((C, C), np.float32) * s,
        "bv": np.zeros(C, np.float32),
        "Wp": rng.standard_normal((C, C), np.float32) * s,
        "bp": np.zeros(C, np.float32),
    }
    out_y, out_att = kernel(**ins)
    print("y", out_y.shape, out_y.dtype, "att", out_att.shape, out_att.dtype)


# revision 35
# speedup vs baseline: 1.4286x; 1.0010x over previous
"""Causal self-attention (B=1, T=2048, C=1024, H=16) on 8 trn2 NeuronCores.

Sharding: tensor-parallel over heads — 2 heads per core. Each core computes
Q/K/V projections for its head pair from the full (replicated) x, interleaved
RoPE, causal softmax attention (the full post-softmax attention matrix is an
output), and its partial contribution to the output projection
(head_slice @ Wp_rows). The host sums the 8 partial y's and assembles att.

Device compute is bf16 matmul inputs / fp32 PSUM accumulation; softmax is
fp32 (exp on ScalarE with fused per-row accumulation).

Layout notes:
 - x is fed transposed (xT, [C, T]) so it serves as the matmul moving operand
   for all three projections.
 - Q/K are produced d-major (QT/KT: [d, T]) with the head-dim columns of
   Wq/Wk permuted to [evens | odds] so interleaved RoPE becomes two
   partition-block copies + mul/add with host-built cos/sin tables (sign of
   sin pre-folded).
 - Attention scores are built twice: q-major (S) for the softmax/att output,
   and k-major (S^T) whose exp (PT, bf16) feeds P@V. Both are bitwise the
   same product, so the two paths are consistent.
 - P@V uses PT as the stationary operand giving Y q-major; an appended
   ones-column of V gives the row-sum r in the same matmuls. Y is normalized
   per-partition by 1/r, PE-transposed back to d-major, and the output
   projection runs as a single K=128 matmul per row block.
"""

import math

import numpy as np

B, T, C, H = 1, 2048, 1024, 16
D = C // H  # 64
NCORES = 8
HPC = H // NCORES  # heads per core = 2
NB = T // 128  # 16 row blocks
SCALE = 1.0 / math.sqrt(D)
MASK_NEG = -30000.0

_CACHE = {}


def _build_nc(ablate=()):
    import concourse.mybir as mybir
    import concourse.tile as tile
    from concourse import bacc

    f32 = mybir.dt.float32
    f16 = mybir.dt.float16
    EXP = mybir.ActivationFunctionType.Exp

    nc = bacc.Bacc("TRN2", target_bir_lowering=False, debug=False, num_devices=NCORES)

    xT = nc.dram_tensor("xT", [C, T], f16, kind="ExternalInput").ap()
    wq = nc.dram_tensor("wq", [C, 128], f16, kind="ExternalInput").ap()
    wk = nc.dram_tensor("wk", [C, 128], f16, kind="ExternalInput").ap()
    wv = nc.dram_tensor("wv", [C, 128], f16, kind="ExternalInput").ap()
    wp = nc.dram_tensor("wp", [128, C], f16, kind="ExternalInput").ap()
    bq = nc.dram_tensor("bq", [128, 1], f32, kind="ExternalInput").ap()
    bk = nc.dram_tensor("bk", [128, 1], f32, kind="ExternalInput").ap()
    cosT = nc.dram_tensor("cosT", [128, T], f16, kind="ExternalInput").ap()
    sinT = nc.dram_tensor("sinT", [128, T], f16, kind="ExternalInput").ap()
    maskA = nc.dram_tensor("maskA", [128, 128], f16, kind="ExternalInput").ap()
    maskAT = nc.dram_tensor("maskAT", [128, 128], f16, kind="ExternalInput").ap()
    ident = nc.dram_tensor("ident", [128, 128], f16, kind="ExternalInput").ap()
    identf = nc.dram_tensor("identf", [128, 128], f32, kind="ExternalInput").ap()

    att = nc.dram_tensor("att", [HPC, T, T], f16, kind="ExternalOutput").ap()
    y = nc.dram_tensor("y", [T, C], f32, kind="ExternalOutput").ap()

    with tile.TileContext(nc) as tc:
        with (
            tc.tile_pool(name="persist", bufs=1) as persist,
            tc.tile_pool(name="sp", bufs=3, space="PSUM") as sp,
            tc.tile_pool(name="psY", bufs=2, space="PSUM") as psY,
            tc.tile_pool(name="attA", bufs=4) as attA,
            tc.tile_pool(name="ysc", bufs=2) as ysc,
            tc.tile_pool(name="small", bufs=8) as small,
            tc.tile_pool(name="yo", bufs=2) as yop,
            tc.tile_pool(name="rope_tmp", bufs=2) as ropep,
        ):
            # ---- resident SBUF tensors ----
            wp_sb = persist.tile([128, C], f16, tag="wp")
            bq_sb = persist.tile([128, 1], f32, tag="bq")
            bk_sb = persist.tile([128, 1], f32, tag="bk")
            cos_sb = persist.tile([128, T], f16, tag="cos")
            sin_sb = persist.tile([128, T], f16, tag="sin")
            mA_sb = persist.tile([128, 128], f16, tag="mA")
            mAT_sb = persist.tile([128, 128], f16, tag="mAT")
            id_sb = persist.tile([128, 128], f16, tag="id")
            idf_sb = persist.tile([128, 128], f32, tag="idf")
            qt_sb = persist.tile([128, T], f16, tag="qt")
            kt_sb = persist.tile([128, T], f16, tag="kt")
            v_sb = persist.tile([128, NB, 2 * (D + 1)], f16, tag="v")
            yt_sb = persist.tile([128, T], f16, tag="yt")

            # warm up the ACT exp table while input DMAs run
            wu = small.tile([1, 8], f32, tag="wu")
            wu2 = small.tile([1, 8], f32, tag="wu2")
            nc.vector.memset(wu, 0.0)
            nc.scalar.activation(wu2, wu, EXP)

            xT_sb = persist.tile([128, 8, T], f16, tag="xT")
            wq_sb = persist.tile([128, 8, 128], f16, tag="wq")
            wk_sb = persist.tile([128, 8, 128], f16, tag="wk")
            wv_sb = persist.tile([128, 8, 128], f16, tag="wv")

            # weights for Q/K first — they gate the first matmuls
            nc.sync.dma_start(out=wq_sb, in_=wq.rearrange("(c p) d -> p c d", p=128))
            nc.sync.dma_start(out=wk_sb, in_=wk.rearrange("(c p) d -> p c d", p=128))
            nc.sync.dma_start(out=bq_sb, in_=bq)
            nc.sync.dma_start(out=bk_sb, in_=bk)
            nc.sync.dma_start(out=cos_sb, in_=cosT)
            nc.sync.dma_start(out=sin_sb, in_=sinT)
            xTr = xT.rearrange("(c p) t -> p c t", p=128)
            for cc in range(8):
                nc.sync.dma_start(out=xT_sb[:, cc, :], in_=xTr[:, cc, :])
            nc.sync.dma_start(out=mA_sb, in_=maskA)
            nc.sync.dma_start(out=mAT_sb, in_=maskAT)
            nc.sync.dma_start(out=id_sb, in_=ident)
            nc.sync.dma_start(out=idf_sb, in_=identf)
            nc.sync.dma_start(out=wv_sb, in_=wv.rearrange("(c p) d -> p c d", p=128))
            nc.sync.dma_start(out=wp_sb, in_=wp)

            # ones columns of the augmented V (for the P@V row-sum trick)
            nc.vector.memset(v_sb[:, :, D], 1.0)
            nc.vector.memset(v_sb[:, :, 2 * D + 1], 1.0)

            # ========= Q/K projections + RoPE, in 512-col t-chunks so
            # ========= attention on early row-blocks starts ASAP
            for tch in range(4):
                t0 = tch * 512
                ts = slice(t0, t0 + 512)
                for dst, w_sb, b_sb in ((qt_sb, wq_sb, bq_sb), (kt_sb, wk_sb, bk_sb)):
                    ps = sp.tile([128, 1024], f32, tag="s")
                    for cc in range(8):
                        nc.tensor.matmul(
                            ps[:, 0:512],
                            lhsT=w_sb[:, cc, :],
                            rhs=xT_sb[:, cc, ts],
                            start=(cc == 0),
                            stop=(cc == 7),
                        )
                    # bias-add + cast on ScalarE — ACT is idle during
                    # startup; keeps the PSUM slot rotation off DVE
                    nc.scalar.add(dst[:, ts], ps[:, 0:512], b_sb)

                for tgt in (qt_sb, kt_sb):
                    tmp = ropep.tile([128, 512], f16, tag="rope")
                    for h in range(2):
                        o = h * 64
                        nc.vector.tensor_copy(tmp[o:o + 32, :], tgt[o + 32:o + 64, ts])
                        nc.vector.tensor_copy(tmp[o + 32:o + 64, :], tgt[o:o + 32, ts])
                    nc.vector.tensor_mul(tgt[:, ts], tgt[:, ts], cos_sb[:, ts])
                    nc.vector.tensor_mul(tmp, tmp, sin_sb[:, ts])
                    nc.vector.tensor_add(tgt[:, ts], tgt[:, ts], tmp)

            def emit_V(kb):
                if 'V' in ablate:
                    return
                pv = sp.tile([128, 1024], f32, tag="s")
                for cc in range(8):
                    nc.tensor.matmul(
                        pv[:, 0:128],
                        lhsT=xT_sb[:, cc, kb * 128:(kb + 1) * 128],
                        rhs=wv_sb[:, cc, :],
                        start=(cc == 0),
                        stop=(cc == 7),
                    )
                nc.vector.tensor_copy(
                    v_sb[:, kb, :].rearrange("p (g x) -> p g x", g=2)[:, :, 0:D],
                    pv[:, 0:128].rearrange("p (g x) -> p g x", g=2),
                )

            def emit_A(h, qb):
                if 'A' in ablate:
                    return
                hs = slice(h * 64, (h + 1) * 64)
                klen = (qb + 1) * 128
                a_t = attA.tile([128, T], f16, tag="att")
                rparts = []
                for ch in range((klen + 1023) // 1024):
                    c0, c1 = ch * 1024, min(ch * 1024 + 1024, klen)
                    ps = sp.tile([128, 1024], f32, tag="s")
                    for m0 in range(c0, c1, 512):
                        m1 = min(m0 + 512, c1)
                        last = m1 == klen
                        nc.tensor.matmul(
                            ps[:, m0 - c0:m1 - c0],
                            lhsT=qt_sb[hs, qb * 128:(qb + 1) * 128],
                            rhs=kt_sb[hs, m0:m1],
                            start=True,
                            stop=not last,
                        )
                        if last:
                            nc.tensor.matmul(
                                ps[:, klen - 128 - c0:klen - c0],
                                lhsT=id_sb,
                                rhs=mA_sb,
                                start=False,
                                stop=True,
                            )
                    r_c = small.tile([128, 1], f32, tag="r")
                    nc.scalar.activation(
                        a_t[:, c0:c1], ps[:, 0:c1 - c0], EXP, scale=SCALE,
                        accum_out=r_c,
                    )
                    rparts.append(r_c)
                if len(rparts) == 2:
                    rsum = small.tile([128, 1], f32, tag="r")
                    nc.vector.tensor_add(rsum, rparts[0], rparts[1])
                else:
                    rsum = rparts[0]
                inv_t = small.tile([128, 1], f32, tag="inv")
                nc.vector.reciprocal(inv_t, rsum)
                # normalization split across GpSimd (idle) and DVE so the
                # halves run in parallel and the latency on the critical
                # path is halved
                if klen > 512:
                    half = (klen // 2 + 127) & ~127
                    nc.gpsimd.tensor_scalar_mul(
                        a_t[:, 0:half], a_t[:, 0:half], inv_t
                    )
                    nc.vector.tensor_scalar_mul(
                        a_t[:, half:klen], a_t[:, half:klen], inv_t
                    )
                else:
                    nc.vector.tensor_scalar_mul(a_t[:, 0:klen], a_t[:, 0:klen], inv_t)
                if 'adma' not in ablate:
                    nc.sync.dma_start(
                        out=att[h, qb * 128:(qb + 1) * 128, 0:klen],
                        in_=a_t[:, 0:klen],
                    )

            def emit_B(h, qb, pt_strips, ptp):
                if 'B' in ablate:
                    return
                hs = slice(h * 64, (h + 1) * 64)
                qlen = T - qb * 128
                pt_t = ptp.tile([128, qlen], f16, tag=f"pt{qb}")
                for sch in range((qlen + 1023) // 1024):
                    s0, s1 = sch * 1024, min(sch * 1024 + 1024, qlen)
                    pb = sp.tile([128, 1024], f32, tag="s")
                    for m0 in range(s0, s1, 512):
                        m1 = min(m0 + 512, s1)
                        first = m0 == 0
                        nc.tensor.matmul(
                            pb[:, m0 - s0:m1 - s0],
                            lhsT=kt_sb[hs, qb * 128:(qb + 1) * 128],
                            rhs=qt_sb[hs, qb * 128 + m0:qb * 128 + m1],
                            start=True,
                            stop=not first,
                        )
                        if first:
                            nc.tensor.matmul(
                                pb[:, 0:128],
                                lhsT=id_sb,
                                rhs=mAT_sb,
                                start=False,
                                stop=True,
                            )
                    nc.scalar.activation(
                        pt_t[:, s0:s1], pb[:, 0:s1 - s0], EXP, scale=SCALE
                    )
                pt_strips.append(pt_t)

                # Y[qb] = sum_kb PT[kb][:, qb].T @ [V|1]; kb descending so the
                # freshest strip (kb==qb, still in flight on ACT) comes first
                # and the PSUM accumulator isn't held across its exp latency.
                # The tile has spare columns used as the transpose target —
                # the two accumulation groups are sequential in the same bank.
                py = psY.tile([128, D + 1 + 128], f32, tag="y")
                for i, kb in enumerate(range(qb, -1, -1)):
                    nc.tensor.matmul(
                        py[:, 0:D + 1],
                        lhsT=pt_strips[kb][:, (qb - kb) * 128:(qb - kb + 1) * 128],
                        rhs=v_sb[:, kb, h * (D + 1):(h + 1) * (D + 1)],
                        start=(i == 0),
                        stop=(i == qb),
                    )
                rinv = small.tile([128, 1], f32, tag="rinv")
                nc.vector.reciprocal(rinv, py[:, D:D + 1])
                ysb = ysc.tile([128, D], f32, tag="ysb")
                nc.vector.tensor_scalar_mul(ysb, py[:, 0:D], rinv)
                nc.tensor.transpose(py[0:D, D + 1:D + 1 + 128], ysb, idf_sb)
                nc.vector.tensor_copy(
                    yt_sb[hs, qb * 128:(qb + 1) * 128], py[0:D, D + 1:D + 1 + 128]
                )

            def emit_proj(tb):
                if 'P' in ablate:
                    return
                yo = yop.tile([128, 1024], f32, tag="yo")
                for nch in range(2):
                    pp = sp.tile([128, 1024], f32, tag="s")
                    nc.tensor.matmul(
                        pp[:, 0:512],
                        lhsT=yt_sb[:, tb * 128:(tb + 1) * 128],
                        rhs=wp_sb[:, nch * 512:(nch + 1) * 512],
                        start=True,
                        stop=True,
                    )
                    nc.vector.tensor_copy(yo[:, nch * 512:(nch + 1) * 512], pp[:, 0:512])
                nc.sync.dma_start(out=y[tb * 128:(tb + 1) * 128, :], in_=yo)

            # ==== interleaved attention (both heads) / V / projection ====
            with (
                tc.tile_pool(name="ptp0", bufs=1) as ptp0,
                tc.tile_pool(name="ptp1", bufs=1) as ptp1,
            ):
                strips = {0: [], 1: []}
                ptps = {0: ptp0, 1: ptp1}
                # A-order: small blocks first (only need the first RoPE'd
                # t-half), then largest-first to front-load the att DMA
                a_order = list(range(8)) + list(range(NB - 1, 7, -1))
                # proj lags 4 iterations so its PSUM-slot reuse never couples
                # the next iteration's work to this iteration's tail
                PROJ_LAG = 2
                emit_V(0)
                for i in range(NB):
                    emit_A(0, a_order[i])
                    emit_A(1, a_order[i])
                    if i + 1 < NB:
                        emit_V(i + 1)
                    emit_B(0, i, strips[0], ptps[0])
                    emit_B(1, i, strips[1], ptps[1])
                    if i >= PROJ_LAG:
                        emit_proj(i - PROJ_LAG)
                for tb in range(NB - PROJ_LAG, NB):
                    emit_proj(tb)

    nc.compile()
    return nc


def _perm_cols(h):
    base = h * D
    return np.concatenate([base + np.arange(0, D, 2), base + np.arange(1, D, 2)])


def _rope_tables():
    # permuted layout: per head, partitions [0:32) = even dims, [32:64) = odd
    div = np.exp(np.arange(0, D, 2, dtype=np.float64) * (-math.log(10000.0) / D))
    ang = np.arange(T, dtype=np.float64)[None, :] * div[:, None]  # [32, T]
    cos32 = np.cos(ang)
    sin32 = np.sin(ang)
    cos64 = np.concatenate([cos32, cos32], axis=0)  # [64, T]
    sin64 = np.concatenate([-sin32, sin32], axis=0)  # evens get -sin
    cosP = np.concatenate([cos64, cos64], axis=0).astype(np.float32)  # [128, T]
    sinP = np.concatenate([sin64, sin64], axis=0).astype(np.float32)
    return cosP, sinP


def kernel(x, Wq, bq, Wk, bk, Wv, bv, Wp, bp):
    import ml_dtypes
    from concourse.bass_utils import run_bass_kernel_spmd

    bf16 = ml_dtypes.bfloat16

    if "nc" not in _CACHE:
        _CACHE["nc"] = _build_nc()
    nc = _CACHE["nc"]

    x = np.asarray(x, np.float32)
    Wq, Wk, Wv, Wp = (np.asarray(w, np.float32) for w in (Wq, Wk, Wv, Wp))
    bq, bk, bv, bp = (np.asarray(b, np.float32) for b in (bq, bk, bv, bp))

    f16 = np.float16
    xT = np.ascontiguousarray(x[0].T).astype(f16)
    cosP, sinP = _rope_tables()
    qk = np.arange(128)
    maskA = np.where(qk[None, :] <= qk[:, None], 0.0, MASK_NEG).astype(np.float32)

    common = {
        "xT": xT,
        "cosT": cosP.astype(f16),
        "sinT": sinP.astype(f16),
        "maskA": maskA.astype(f16),
        "maskAT": np.ascontiguousarray(maskA.T).astype(f16),
        "ident": np.eye(128, dtype=f16),
        "identf": np.eye(128, dtype=np.float32),
    }
    in_maps = []
    for c in range(NCORES):
        cols = np.concatenate([_perm_cols(2 * c), _perm_cols(2 * c + 1)])
        rows = slice(2 * c * D, (2 * c + 2) * D)
        in_maps.append(
            dict(
                common,
                wq=np.ascontiguousarray(Wq[:, cols]).astype(f16),
                wk=np.ascontiguousarray(Wk[:, cols]).astype(f16),
                wv=np.ascontiguousarray(Wv[:, rows]).astype(f16),
                wp=np.ascontiguousarray(Wp[rows, :]).astype(f16),
                bq=np.ascontiguousarray(bq[cols]).reshape(128, 1),
                bk=np.ascontiguousarray(bk[cols]).reshape(128, 1),
            )
        )

    res = run_bass_kernel_spmd(nc, in_maps, core_ids=list(range(NCORES)))

    att_full = np.zeros((B, H, T, T), np.float32)
    y_full = np.zeros((T, C), np.float32)
    for c in range(NCORES):
        out = res.results[c]
        y_full += out["y"]
        for hl in range(HPC):
            a = out["att"][hl]
            dst = att_full[0, 2 * c + hl]
            for qb in range(NB):
                klen = (qb + 1) * 128
                dst[qb * 128:(qb + 1) * 128, 0:klen] = a[
                    qb * 128:(qb + 1) * 128, 0:klen
                ].astype(np.float32)

    # biases folded on host: V bias contributes (sum_k att)=1 times bv per row
    y_full += bv @ Wp + bp
    return y_full.reshape(B, T, C), att_full


if __name__ == "__main__":
    rng = np.random.default_rng(0)
    s = 1.0 / math.sqrt(C)
    ins = {
        "x": rng.standard_normal((B, T, C), np.float32),
        "Wq": rng.standard_normal((C, C), np.float32) * s,
        "bq": np.zeros(C, np.float32),
        "Wk": rng.standard_normal((C, C), np.float32) * s,
        "bk": np.zeros(C, np.float32),
        "Wv": rng.standard_normal((C, C), np.float32) * s,
        "bv": np.zeros(C, np.float32),
        "Wp": rng.standard_normal((C, C), np.float32) * s,
        "bp": np.zeros(C, np.float32),
    }
    out_y, out_att = kernel(**ins)
    print("y", out_y.shape, out_y.dtype, "att", out_att.shape, out_att.dtype)
